# revision 41
# baseline (speedup 1.0000x reference)
"""Two-launch expert-parallel MoE kernel (v9).

Launch 1 (expert-parallel): core e holds expert e's weights (12.6MB bf16).
Host gathers each expert's routed tokens (top-2 routing decided on host by
argsort of f32 logits; pure data placement) into a compact [CAP, D] shard.
Dense SwiGLU FFN with FD=512 matmuls -> compact y [CAP, D] bf16.

Launch 2 (token-parallel): core c owns tokens [512c, 512c+512). Inputs: the
1024 y-rows relevant to its tokens (contiguous per-expert ranges of the
compact outputs, sliced on host), plus x^T for the router. Device computes
router logits, softmax weights of the host-selected top-2 (selection via
one-hot masks; values from device logits), scales y rows, scatters via
one-hot matmul, LayerNorm, writes [512, D] f32.

All model arithmetic (router matmul, softmax, FFN, combine, LN) runs on
device; the host only computes routing indices for data placement.
"""

import numpy as np
import ml_dtypes

P = 128
D_MODEL = 1024
D_FFN = 2048
N_EXPERTS = 8
B, S = 2, 2048
T_FULL = B * S
N_CORES = 8
TC = T_FULL // N_CORES      # 512 tokens per core in launch 2
ROWS = 2 * TC               # 1024 (token, expert) pairs per core in launch 2
DT = D_MODEL // P           # 8
FT = D_FFN // P             # 16
LN_EPS = 1e-5
CAP_DEFAULT = 1072          # max expert load rounded up to 8 (this input: 1071)

_CACHED = {}
TT_L2 = TC // P


def _mm1_chunks(cap):
    """mm1 slot chunks: small first chunk (fast DMA prefix), rest balanced,
    each <=512 and a multiple of 8."""
    first = min(256, cap)
    sizes = [first]
    rem = cap - first
    n_rest = -(-rem // 512) if rem else 0
    for i in range(n_rest):
        s = min(512, -(-rem // (n_rest - i)))
        s = -(-s // 8) * 8 if i < n_rest - 1 else rem
        sizes.append(s)
        rem -= s
    out = []
    c0 = 0
    for s in sizes:
        out.append((c0, s))
        c0 += s
    return len(sizes), out


# --------------------------------------------------------------------------
# Launch 1: dense per-expert SwiGLU FFN on gathered tokens
# --------------------------------------------------------------------------
def _build_l1(cap):
    import concourse.bacc as bacc
    import concourse.mybir as mybir
    import concourse.tile as tile
    import concourse.bass as bass

    f32 = mybir.dt.float32
    bf16 = mybir.dt.bfloat16
    AF = mybir.ActivationFunctionType
    OP = mybir.AluOpType
    AX = mybir.AxisListType
    TT = TC // P  # 4

    nck1, ck1 = _mm1_chunks(cap)

    nc = bacc.Bacc()
    # partition-major host layouts: each dram row = one SBUF partition's bytes
    xgt = nc.dram_tensor("xgt", [P, DT * cap], bf16, kind="ExternalInput")
    wgt = nc.dram_tensor("wgt", [P, 4 * DT * 512], bf16, kind="ExternalInput")
    wut = nc.dram_tensor("wut", [P, 4 * DT * 512], bf16, kind="ExternalInput")
    wdt = nc.dram_tensor("wdt", [P, FT * D_MODEL], bf16, kind="ExternalInput")
    # router inputs for this core's token block (all partition-major)
    xtf = nc.dram_tensor("xtf", [P, DT * TC], bf16, kind="ExternalInput")
    wrt = nc.dram_tensor("wrt", [P, DT * N_EXPERTS], bf16, kind="ExternalInput")
    mmh = nc.dram_tensor("mmh", [N_EXPERTS, 2 * TC], f32, kind="ExternalInput")
    y = nc.dram_tensor("y", [P, DT * cap], bf16, kind="ExternalOutput")
    w12 = nc.dram_tensor("w12", [1, 2 * TC], f32, kind="ExternalOutput")

    wgt_4 = wgt.rearrange("p (hs dt f) -> p hs dt f", hs=8, dt=DT)
    wut_4 = wut.rearrange("p (hs dt f) -> p hs dt f", hs=8, dt=DT)
    wdt_3 = wdt.rearrange("p (ft d) -> p ft d", ft=FT)
    xtf_3 = xtf.rearrange("p (dt t) -> p dt t", dt=DT)
    wrt_3 = wrt.rearrange("p (dt e) -> p dt e", dt=DT)
    y_3 = y.rearrange("p (dt c) -> p dt c", dt=DT)

    with tile.TileContext(nc) as tc:
        with (
            tc.tile_pool(name="xp", bufs=1) as xp,
            tc.tile_pool(name="wp", bufs=2) as wp,
            tc.tile_pool(name="wdp", bufs=1) as wdp,
            tc.tile_pool(name="hp", bufs=1) as hp,
            tc.tile_pool(name="sgp", bufs=2) as sgp,
            tc.tile_pool(name="yp", bufs=2) as yp,
            tc.tile_pool(name="ps", bufs=8, space="PSUM") as ps,
        ):
            # Large batched DMAs with critical prefixes first:
            # router inputs -> wg slab 0 / xg chunk 0 / wu slab 0 -> rest -> wd.
            wg_sb = wp.tile([P, 8, DT, 256], bf16, tag="wg", bufs=1)
            wu_sb = wp.tile([P, 8, DT, 256], bf16, tag="wu", bufs=1)
            xg_sb = xp.tile([P, nck1, DT, 512], bf16)
            rowlen = DT * cap

            def xg_block(ci):
                c0, cw = ck1[ci]
                return bass.AP(
                    tensor=xgt.ap().tensor, offset=DT * c0,
                    ap=[[rowlen, P], [cw, DT], [1, cw]],
                )

            nc.sync.dma_start(out=wg_sb[:, 0], in_=wgt_4[:, 0])
            nc.sync.dma_start(out=xg_sb[:, 0, :, 0 : ck1[0][1]], in_=xg_block(0))
            nc.sync.dma_start(out=wu_sb[:, 0], in_=wut_4[:, 0])
            for ci in range(1, nck1):
                nc.sync.dma_start(out=xg_sb[:, ci, :, 0 : ck1[ci][1]], in_=xg_block(ci))
            nc.sync.dma_start(out=wg_sb[:, 1], in_=wgt_4[:, 1])
            nc.sync.dma_start(out=wu_sb[:, 1], in_=wut_4[:, 1])
            xf_sb = xp.tile([P, DT, TC], bf16, tag="xf")
            nc.sync.dma_start(out=xf_sb, in_=xtf_3)
            wr_sb = xp.tile([P, DT, N_EXPERTS], bf16, tag="wr")
            nc.sync.dma_start(out=wr_sb, in_=wrt_3)
            mm_sb = xp.tile([N_EXPERTS, 2 * TC], f32, tag="mm")
            nc.sync.dma_start(out=mm_sb, in_=mmh.ap())
            for hs in range(2, 8):
                nc.sync.dma_start(out=wg_sb[:, hs], in_=wgt_4[:, hs])
                nc.sync.dma_start(out=wu_sb[:, hs], in_=wut_4[:, hs])
            wd_sb = wdp.tile([P, FT, D_MODEL], bf16)
            nc.sync.dma_start(out=wd_sb, in_=wdt_3)
            h_sb = hp.tile([P, FT, cap], bf16)

            # ---- mm1 + SwiGLU
            for ft in range(FT):
                hs, f2 = divmod(ft, 2)
                for ci, (c0, cw) in enumerate(ck1):
                    pg = ps.tile([P, 512], f32, tag="pg", bufs=2)
                    pu = ps.tile([P, 512], f32, tag="pu", bufs=2)
                    for dt in range(DT):
                        nc.tensor.matmul(
                            pg[:, :cw],
                            lhsT=wg_sb[:, hs, dt, f2 * P : (f2 + 1) * P],
                            rhs=xg_sb[:, ci, dt, 0:cw],
                            start=(dt == 0), stop=(dt == DT - 1),
                        )
                    for dt in range(DT):
                        nc.tensor.matmul(
                            pu[:, :cw],
                            lhsT=wu_sb[:, hs, dt, f2 * P : (f2 + 1) * P],
                            rhs=xg_sb[:, ci, dt, 0:cw],
                            start=(dt == 0), stop=(dt == DT - 1),
                        )
                    sg = sgp.tile([P, 512], f32, tag="sg")
                    nc.scalar.activation(sg[:, :cw], pg[:, :cw], AF.Silu)
                    nc.vector.tensor_mul(
                        h_sb[:, ft, c0 : c0 + cw], sg[:, :cw], pu[:, :cw]
                    )

            # ---- router for this core's token block (wedged between mm1 and mm2):
            # logits + softmax weights of the host-selected top-2 -> w12 [2, TC].
            # Everything stays in [expert, token] orientation; the partition-dim
            # reduction over the 8 experts is a ones-vector matmul.
            ones8 = sgp.tile([N_EXPERTS, 1], f32, tag="ones8", bufs=1)
            nc.vector.memset(ones8, 1.0)
            plT = ps.tile([N_EXPERTS, TC], f32, tag="pg", bufs=2)
            for dt in range(DT):
                nc.tensor.matmul(
                    plT, lhsT=wr_sb[:, dt, :], rhs=xf_sb[:, dt, :],
                    start=(dt == 0), stop=(dt == DT - 1),
                )
            prod1 = sgp.tile([N_EXPERTS, TC], f32, tag="prod1", bufs=1)
            nc.vector.tensor_mul(prod1, plT, mm_sb[:, 0:TC])
            prod2 = sgp.tile([N_EXPERTS, TC], f32, tag="prod2", bufs=1)
            nc.vector.tensor_mul(prod2, plT, mm_sb[:, TC : 2 * TC])
            plv1 = ps.tile([1, TC], f32, tag="pg", bufs=2)
            nc.tensor.matmul(plv1, lhsT=ones8, rhs=prod1, start=True, stop=True)
            plv2 = ps.tile([1, TC], f32, tag="pu", bufs=2)
            nc.tensor.matmul(plv2, lhsT=ones8, rhs=prod2, start=True, stop=True)
            lv1 = sgp.tile([1, TC], f32, tag="lv1", bufs=1)
            nc.vector.tensor_copy(lv1, plv1)
            d21 = sgp.tile([1, TC], f32, tag="d21", bufs=1)
            nc.vector.tensor_sub(d21, plv2, lv1)
            ex = sgp.tile([1, TC], f32, tag="ex", bufs=1)
            nc.scalar.activation(ex, d21, AF.Exp)
            den = sgp.tile([1, TC], f32, tag="den", bufs=1)
            nc.vector.tensor_scalar(den, ex, scalar1=1.0, scalar2=None, op0=OP.add)
            w12T = sgp.tile([1, 2 * TC], f32, tag="w12T", bufs=1)
            nc.vector.reciprocal(w12T[:, 0:TC], den)
            nc.vector.tensor_mul(w12T[:, TC : 2 * TC], ex, w12T[:, 0:TC])
            nc.sync.dma_start(out=w12.ap(), in_=w12T)

            # ---- mm2 (transposed): yT[d, slot] = sum_f wd[f, d] * h[f, slot]
            yT_sb = yp.tile([P, DT, cap], bf16, tag="yT", bufs=1)
            for dt in range(DT):
                for ci, (c0, cw) in enumerate(ck1):
                    pyt = ps.tile([P, 512], f32, tag="py", bufs=2)
                    for ft in range(FT):
                        nc.tensor.matmul(
                            pyt[:, :cw],
                            lhsT=wd_sb[:, ft, dt * P : (dt + 1) * P],
                            rhs=h_sb[:, ft, c0 : c0 + cw],
                            start=(ft == 0), stop=(ft == FT - 1),
                        )
                    nc.vector.tensor_copy(yT_sb[:, dt, c0 : c0 + cw], pyt[:, :cw])
                nc.sync.dma_start(out=y_3[:, dt, :], in_=yT_sb[:, dt, :])

    nc.finalize()
    return nc


# --------------------------------------------------------------------------
# Launch 2: elementwise combine + LayerNorm. The host orders the y rows as
# two token-ordered blocks (top-1 rows, top-2 rows) so the combine is
# out[t] = w1[t]*y1[t] + w2[t]*y2[t] -- no scatter matmuls needed. All values
# (y, w1, w2) are device-computed in L1; the host only permutes them.
# --------------------------------------------------------------------------
def _build_l2(affine):
    import concourse.bacc as bacc
    import concourse.mybir as mybir
    import concourse.tile as tile

    f32 = mybir.dt.float32
    bf16 = mybir.dt.bfloat16
    AF = mybir.ActivationFunctionType
    OP = mybir.AluOpType

    TT = TC // P  # 4 token tiles

    nc = bacc.Bacc()
    yct = nc.dram_tensor("yct", [P, 2 * TT * D_MODEL], bf16, kind="ExternalInput")
    meta = nc.dram_tensor("meta", [P, 2 * TT], f32, kind="ExternalInput")
    if affine:
        gam = nc.dram_tensor("gam", [D_MODEL], bf16, kind="ExternalInput")
        bet = nc.dram_tensor("bet", [D_MODEL], bf16, kind="ExternalInput")
    out = nc.dram_tensor("out", [P, TT * D_MODEL], f32, kind="ExternalOutput")

    yct_3 = yct.rearrange("p (rc d) -> p rc d", rc=2 * TT)
    out_3 = out.rearrange("p (tt d) -> p tt d", tt=TT)

    with tile.TileContext(nc) as tc:
        with (
            tc.tile_pool(name="consts", bufs=1) as consts,
            tc.tile_pool(name="rtr", bufs=2) as rtr,
            tc.tile_pool(name="ycp", bufs=1) as ycp,
            tc.tile_pool(name="outp", bufs=1) as outp,
        ):
            meta_sb = consts.tile([P, 2 * TT], f32)
            nc.sync.dma_start(out=meta_sb, in_=meta.ap())
            yc_sb = ycp.tile([P, 2 * TT, D_MODEL], bf16)
            for tt in range(TT):  # tt-block pairs first so tt=0 starts earliest
                nc.sync.dma_start(out=yc_sb[:, tt, :], in_=yct_3[:, tt, :])
                nc.sync.dma_start(out=yc_sb[:, TT + tt, :], in_=yct_3[:, TT + tt, :])
            if affine:
                import concourse.bass as bass
                gam_sb = consts.tile([P, D_MODEL], bf16)
                bet_sb = consts.tile([P, D_MODEL], bf16)
                nc.sync.dma_start(
                    out=gam_sb,
                    in_=bass.AP(tensor=gam.ap().tensor, offset=0, ap=[[0, P], [1, D_MODEL]]),
                )
                nc.sync.dma_start(
                    out=bet_sb,
                    in_=bass.AP(tensor=bet.ap().tensor, offset=0, ap=[[0, P], [1, D_MODEL]]),
                )
            eps_sb = consts.tile([P, 1], f32)
            nc.vector.memset(eps_sb, LN_EPS)

            # per-tt chain spread across GpSimd (mul), DVE (fused mul-add +
            # row-sum), and ACT (square-sum, rsqrt, normalize)
            o_sb = outp.tile([P, TT, D_MODEL], f32, tag="acc", bufs=1)
            inv_d = 1.0 / D_MODEL
            for tt in range(TT):
                t1 = rtr.tile([P, D_MODEL], f32, tag="t1")
                nc.gpsimd.tensor_scalar(
                    t1, yc_sb[:, tt, :], scalar1=meta_sb[:, tt : tt + 1],
                    scalar2=None, op0=OP.mult,
                )
                a = o_sb[:, tt, :]
                ssum = rtr.tile([P, 1], f32, tag="ssum")
                nc.vector.scalar_tensor_tensor(
                    a, yc_sb[:, TT + tt, :], meta_sb[:, TT + tt : TT + tt + 1], t1,
                    op0=OP.mult, op1=OP.add, accum_out=ssum,
                )
                sqj = rtr.tile([P, D_MODEL], bf16, tag="sqj")
                ssq = rtr.tile([P, 1], f32, tag="ssq")
                nc.scalar.activation(sqj, a, AF.Square, accum_out=ssq)
                mean = rtr.tile([P, 1], f32, tag="mean")
                nc.vector.tensor_scalar(
                    mean, ssum, scalar1=inv_d, scalar2=None, op0=OP.mult
                )
                aeps = rtr.tile([P, 1], f32, tag="aeps")
                nc.vector.tensor_scalar(
                    aeps, ssq, scalar1=inv_d, scalar2=eps_sb, op0=OP.mult, op1=OP.add
                )
                mean2 = rtr.tile([P, 1], f32, tag="mean2")
                nc.vector.tensor_mul(mean2, mean, mean)
                rstd = rtr.tile([P, 1], f32, tag="rstd")
                nc.scalar.activation(rstd, mean2, AF.Sqrt, bias=aeps, scale=-1.0)
                nc.vector.reciprocal(rstd, rstd)
                nmr = rtr.tile([P, 1], f32, tag="nmr")
                nc.vector.tensor_scalar(
                    nmr, mean, scalar1=-1.0, scalar2=rstd, op0=OP.mult, op1=OP.mult
                )
                of = rtr.tile([P, D_MODEL], f32, tag="of")
                nc.scalar.activation(of, a, AF.Identity, bias=nmr, scale=rstd)
                if affine:
                    nc.vector.tensor_mul(of, of, gam_sb)
                    nc.vector.tensor_add(of, of, bet_sb)
                nc.sync.dma_start(out=out_3[:, tt, :], in_=of)

    nc.finalize()
    return nc


# --------------------------------------------------------------------------
# Host orchestration
# --------------------------------------------------------------------------
def _route(x2, w_router):
    logits = x2 @ w_router.T
    order = np.argsort(-logits, axis=1)
    top1 = order[:, 0].astype(np.int64)
    top2 = order[:, 1].astype(np.int64)
    return top1, top2


def _prepare(inputs):
    bf = ml_dtypes.bfloat16
    x2 = np.ascontiguousarray(
        np.asarray(inputs["x"], dtype=np.float32).reshape(T_FULL, D_MODEL)
    )
    w_router = np.asarray(inputs["w_router"], dtype=np.float32)
    top1, top2 = _route(x2, w_router)

    # per-expert token lists (ascending)
    tok = [np.where((top1 == e) | (top2 == e))[0] for e in range(N_EXPERTS)]
    caps = [len(t) for t in tok]
    cap_needed = max(caps)
    return x2, w_router, top1, top2, tok, caps, cap_needed


def _pm(a, inner, width):
    """[ (g p), w ] row-major -> partition-major [P, g*w] contiguous rows."""
    g = a.shape[0] // P
    return np.ascontiguousarray(
        a.reshape(g, P, inner, width).transpose(1, 0, 2, 3).reshape(P, -1)
        if inner > 1 else
        a.reshape(g, P, width).transpose(1, 0, 2).reshape(P, -1)
    )


def _l1_in_maps(inputs, x2, w_router, top1, top2, tok, cap):
    bf = ml_dtypes.bfloat16
    nck1, ck1 = _mm1_chunks(cap)
    w_gate = np.asarray(inputs["w_gate"], dtype=np.float32)
    w_up = np.asarray(inputs["w_up"], dtype=np.float32)
    w_down = np.asarray(inputs["w_down"], dtype=np.float32)
    # wrt: [d, e] -> [P, dt*e] partition-major
    wrt = np.ascontiguousarray(
        w_router.T.reshape(DT, P, N_EXPERTS).transpose(1, 0, 2).reshape(P, -1)
    ).astype(bf)
    m1_full = np.zeros((N_EXPERTS, T_FULL), np.float32)
    m1_full[top1, np.arange(T_FULL)] = 1.0
    m2_full = np.zeros((N_EXPERTS, T_FULL), np.float32)
    m2_full[top2, np.arange(T_FULL)] = 1.0
    in_maps = []
    for e in range(N_EXPERTS):
        # xg: [P, nck1, DT, 512] partition-major, chunk blocks padded to 512
        xgT = np.zeros((D_MODEL, cap), np.float32)
        xgT[:, : len(tok[e])] = x2[tok[e]].T
        xgT_r = xgT.reshape(DT, P, cap)
        xg4 = np.empty((P, DT * cap), np.float32)
        for ci, (c0, cw) in enumerate(ck1):
            xg4[:, DT * c0 : DT * (c0 + cw)] = (
                xgT_r[:, :, c0 : c0 + cw].transpose(1, 0, 2).reshape(P, -1)
            )
        # wg/wu: [(dt p), f] -> [P, fs, dt, 512] -> rows
        wgT = w_gate[e].T.reshape(DT, P, 8, 256)
        wuT = w_up[e].T.reshape(DT, P, 8, 256)
        wg4 = wgT.transpose(1, 2, 0, 3).reshape(P, -1)
        wu4 = wuT.transpose(1, 2, 0, 3).reshape(P, -1)
        # wd: [(ft p), d] -> [P, ft, d] -> rows
        wd3 = w_down[e].T.reshape(FT, P, D_MODEL).transpose(1, 0, 2).reshape(P, -1)
        lo, hi = e * TC, (e + 1) * TC  # this core also routes token block e
        xf3 = x2[lo:hi].T.reshape(DT, P, TC).transpose(1, 0, 2).reshape(P, -1)
        in_maps.append({
            "xgt": np.ascontiguousarray(xg4).astype(bf),
            "wgt": np.ascontiguousarray(wg4).astype(bf),
            "wut": np.ascontiguousarray(wu4).astype(bf),
            "wdt": np.ascontiguousarray(wd3).astype(bf),
            "xtf": np.ascontiguousarray(xf3).astype(bf),
            "wrt": wrt,
            "mmh": np.ascontiguousarray(
                np.concatenate([m1_full[:, lo:hi], m2_full[:, lo:hi]], axis=1)
            ),
        })
    return in_maps


def _l2_in_maps(inputs, top1, top2, tok, y_parts, w12_parts, affine):
    bf = ml_dtypes.bfloat16
    TT = TC // P
    in_maps = []
    for c in range(N_CORES):
        lo, hi = c * TC, (c + 1) * TC
        y1 = np.empty((TC, D_MODEL), y_parts[0].dtype)
        y2 = np.empty((TC, D_MODEL), y_parts[0].dtype)
        t1c = top1[lo:hi]
        t2c = top2[lo:hi]
        for e in range(N_EXPERTS):
            m = t1c == e
            if m.any():
                y1[m] = y_parts[e][np.searchsorted(tok[e], np.nonzero(m)[0] + lo)]
            m = t2c == e
            if m.any():
                y2[m] = y_parts[e][np.searchsorted(tok[e], np.nonzero(m)[0] + lo)]
        yct = np.concatenate([y1, y2], axis=0)
        meta = np.empty((P, 2 * TT), np.float32)
        meta[:, :TT] = w12_parts[c][0].reshape(TT, P).T
        meta[:, TT:] = w12_parts[c][1].reshape(TT, P).T
        in_map = {
            "yct": np.ascontiguousarray(
                yct.reshape(2 * TT, P, D_MODEL).transpose(1, 0, 2).reshape(P, -1)
            ),
            "meta": meta,
        }
        if affine:
            in_map["gam"] = np.asarray(inputs["ln_gamma"], np.float32).astype(bf)
            in_map["bet"] = np.asarray(inputs["ln_beta"], np.float32).astype(bf)
        in_maps.append(in_map)
    return in_maps


def run_launches(inputs, trace=False):
    from concourse.bass_utils import run_bass_kernel_spmd

    x2, w_router, top1, top2, tok, caps, cap_needed = _prepare(inputs)
    cap = _CACHED.get("cap", CAP_DEFAULT)
    if cap_needed > cap:
        cap = int(-(-cap_needed // 8) * 8)
        _CACHED.pop("l1", None)
    affine = not (
        np.all(np.asarray(inputs["ln_gamma"]) == 1.0)
        and np.all(np.asarray(inputs["ln_beta"]) == 0.0)
    )
    if "l1" not in _CACHED or _CACHED.get("cap") != cap:
        _CACHED["cap"] = cap
        _CACHED["l1"] = _build_l1(cap)
    if "l2" not in _CACHED or _CACHED.get("affine") != affine:
        _CACHED["affine"] = affine
        _CACHED["l2"] = _build_l2(affine)

    l1_maps = _l1_in_maps(inputs, x2, w_router, top1, top2, tok, cap)
    res1 = run_bass_kernel_spmd(
        _CACHED["l1"], l1_maps, core_ids=list(range(N_CORES)), trace=trace
    )
    cap = _CACHED["cap"]
    y_parts = [
        np.asarray(res1.results[e]["y"]).reshape(P, DT, cap).transpose(2, 1, 0).reshape(cap, D_MODEL)
        for e in range(N_EXPERTS)
    ]
    w12_parts = [
        np.asarray(res1.results[c]["w12"]).reshape(2, TC) for c in range(N_CORES)
    ]

    l2_maps = _l2_in_maps(inputs, top1, top2, tok, y_parts, w12_parts, affine)
    res2 = run_bass_kernel_spmd(
        _CACHED["l2"], l2_maps, core_ids=list(range(N_CORES)), trace=trace
    )
    outs = []
    for c in range(N_CORES):
        o = np.asarray(res2.results[c]["out"]).reshape(P, TT_L2, D_MODEL)
        outs.append(o.transpose(1, 0, 2).reshape(TC, D_MODEL))
    out = np.concatenate(outs, axis=0)
    return out.reshape(B, S, D_MODEL), res1, res2


def kernel(**inputs) -> np.ndarray:
    out, _, _ = run_launches(inputs, trace=False)
    return out


# revision 42
# speedup vs baseline: 1.2079x; 1.2079x over previous
"""Two-launch expert-parallel MoE kernel (v9).

Launch 1 (expert-parallel): core e holds expert e's weights (12.6MB bf16).
Host gathers each expert's routed tokens (top-2 routing decided on host by
argsort of f32 logits; pure data placement) into a compact [CAP, D] shard.
Dense SwiGLU FFN with FD=512 matmuls -> compact y [CAP, D] bf16.

Launch 2 (token-parallel): core c owns tokens [512c, 512c+512). Inputs: the
1024 y-rows relevant to its tokens (contiguous per-expert ranges of the
compact outputs, sliced on host), plus x^T for the router. Device computes
router logits, softmax weights of the host-selected top-2 (selection via
one-hot masks; values from device logits), scales y rows, scatters via
one-hot matmul, LayerNorm, writes [512, D] f32.

All model arithmetic (router matmul, softmax, FFN, combine, LN) runs on
device; the host only computes routing indices for data placement.
"""

import numpy as np
import ml_dtypes

P = 128
D_MODEL = 1024
D_FFN = 2048
N_EXPERTS = 8
B, S = 2, 2048
T_FULL = B * S
N_CORES = 8
TC = T_FULL // N_CORES      # 512 tokens per core in launch 2
ROWS = 2 * TC               # 1024 (token, expert) pairs per core in launch 2
DT = D_MODEL // P           # 8
FT = D_FFN // P             # 16
LN_EPS = 1e-5
CAP_DEFAULT = 1072          # max expert load rounded up to 8 (this input: 1071)

_CACHED = {}
TT_L2 = TC // P


def _mm1_chunks(cap):
    """mm1 slot chunks: small first chunk (fast DMA prefix), rest balanced,
    each <=512 and a multiple of 8."""
    first = min(256, cap)
    sizes = [first]
    rem = cap - first
    n_rest = -(-rem // 512) if rem else 0
    for i in range(n_rest):
        s = min(512, -(-rem // (n_rest - i)))
        s = -(-s // 8) * 8 if i < n_rest - 1 else rem
        sizes.append(s)
        rem -= s
    out = []
    c0 = 0
    for s in sizes:
        out.append((c0, s))
        c0 += s
    return len(sizes), out


# --------------------------------------------------------------------------
# Launch 1: dense per-expert SwiGLU FFN on gathered tokens
# --------------------------------------------------------------------------
def _build_l1(cap):
    import concourse.bacc as bacc
    import concourse.mybir as mybir
    import concourse.tile as tile
    import concourse.bass as bass

    f32 = mybir.dt.float32
    bf16 = mybir.dt.bfloat16
    AF = mybir.ActivationFunctionType
    OP = mybir.AluOpType
    AX = mybir.AxisListType
    TT = TC // P  # 4

    nck1, ck1 = _mm1_chunks(cap)

    nc = bacc.Bacc()
    # partition-major host layouts: each dram row = one SBUF partition's bytes
    xgt = nc.dram_tensor("xgt", [P, DT * cap], bf16, kind="ExternalInput")
    wgt = nc.dram_tensor("wgt", [P, 4 * DT * 512], bf16, kind="ExternalInput")
    wut = nc.dram_tensor("wut", [P, 4 * DT * 512], bf16, kind="ExternalInput")
    wdt = nc.dram_tensor("wdt", [P, FT * D_MODEL], bf16, kind="ExternalInput")
    # router inputs for this core's token block (all partition-major)
    xtf = nc.dram_tensor("xtf", [P, DT * TC], bf16, kind="ExternalInput")
    wrt = nc.dram_tensor("wrt", [P, DT * N_EXPERTS], bf16, kind="ExternalInput")
    mmh = nc.dram_tensor("mmh", [N_EXPERTS, 2 * TC], f32, kind="ExternalInput")
    y = nc.dram_tensor("y", [P, DT * cap], bf16, kind="ExternalOutput")
    w12 = nc.dram_tensor("w12", [1, 2 * TC], f32, kind="ExternalOutput")

    wgt_4 = wgt.rearrange("p (hs dt f) -> p hs dt f", hs=8, dt=DT)
    wut_4 = wut.rearrange("p (hs dt f) -> p hs dt f", hs=8, dt=DT)
    wdt_3 = wdt.rearrange("p (ft d) -> p ft d", ft=FT)
    xtf_3 = xtf.rearrange("p (dt t) -> p dt t", dt=DT)
    wrt_3 = wrt.rearrange("p (dt e) -> p dt e", dt=DT)
    y_3 = y.rearrange("p (dt c) -> p dt c", dt=DT)

    with tile.TileContext(nc) as tc:
        with (
            tc.tile_pool(name="xp", bufs=1) as xp,
            tc.tile_pool(name="wp", bufs=2) as wp,
            tc.tile_pool(name="wdp", bufs=1) as wdp,
            tc.tile_pool(name="hp", bufs=1) as hp,
            tc.tile_pool(name="sgp", bufs=2) as sgp,
            tc.tile_pool(name="yp", bufs=2) as yp,
            tc.tile_pool(name="ps", bufs=8, space="PSUM") as ps,
        ):
            # Large batched DMAs with critical prefixes first:
            # router inputs -> wg slab 0 / xg chunk 0 / wu slab 0 -> rest -> wd.
            wg_sb = wp.tile([P, 8, DT, 256], bf16, tag="wg", bufs=1)
            wu_sb = wp.tile([P, 8, DT, 256], bf16, tag="wu", bufs=1)
            xg_sb = xp.tile([P, nck1, DT, 512], bf16)
            rowlen = DT * cap

            def xg_block(ci):
                c0, cw = ck1[ci]
                return bass.AP(
                    tensor=xgt.ap().tensor, offset=DT * c0,
                    ap=[[rowlen, P], [cw, DT], [1, cw]],
                )

            nc.sync.dma_start(out=wg_sb[:, 0], in_=wgt_4[:, 0])
            nc.sync.dma_start(out=xg_sb[:, 0, :, 0 : ck1[0][1]], in_=xg_block(0))
            nc.sync.dma_start(out=wu_sb[:, 0], in_=wut_4[:, 0])
            for ci in range(1, nck1):
                nc.sync.dma_start(out=xg_sb[:, ci, :, 0 : ck1[ci][1]], in_=xg_block(ci))
            nc.sync.dma_start(out=wg_sb[:, 1], in_=wgt_4[:, 1])
            nc.sync.dma_start(out=wu_sb[:, 1], in_=wut_4[:, 1])
            xf_sb = xp.tile([P, DT, TC], bf16, tag="xf")
            nc.sync.dma_start(out=xf_sb, in_=xtf_3)
            wr_sb = xp.tile([P, DT, N_EXPERTS], bf16, tag="wr")
            nc.sync.dma_start(out=wr_sb, in_=wrt_3)
            mm_sb = xp.tile([N_EXPERTS, 2 * TC], f32, tag="mm")
            nc.sync.dma_start(out=mm_sb, in_=mmh.ap())
            for hs in range(2, 8):
                nc.sync.dma_start(out=wg_sb[:, hs], in_=wgt_4[:, hs])
                nc.sync.dma_start(out=wu_sb[:, hs], in_=wut_4[:, hs])
            wd_sb = wdp.tile([P, FT, D_MODEL], bf16)
            nc.sync.dma_start(out=wd_sb, in_=wdt_3)
            h_sb = hp.tile([P, FT, cap], bf16)

            # ---- mm1 + SwiGLU
            for ft in range(FT):
                hs, f2 = divmod(ft, 2)
                for ci, (c0, cw) in enumerate(ck1):
                    pg = ps.tile([P, 512], f32, tag="pg", bufs=2)
                    pu = ps.tile([P, 512], f32, tag="pu", bufs=2)
                    for dt in range(DT):
                        nc.tensor.matmul(
                            pg[:, :cw],
                            lhsT=wg_sb[:, hs, dt, f2 * P : (f2 + 1) * P],
                            rhs=xg_sb[:, ci, dt, 0:cw],
                            start=(dt == 0), stop=(dt == DT - 1),
                        )
                    for dt in range(DT):
                        nc.tensor.matmul(
                            pu[:, :cw],
                            lhsT=wu_sb[:, hs, dt, f2 * P : (f2 + 1) * P],
                            rhs=xg_sb[:, ci, dt, 0:cw],
                            start=(dt == 0), stop=(dt == DT - 1),
                        )
                    sg = sgp.tile([P, 512], f32, tag="sg")
                    nc.scalar.activation(sg[:, :cw], pg[:, :cw], AF.Silu)
                    nc.vector.tensor_mul(
                        h_sb[:, ft, c0 : c0 + cw], sg[:, :cw], pu[:, :cw]
                    )

            # ---- router for this core's token block (wedged between mm1 and mm2):
            # logits + softmax weights of the host-selected top-2 -> w12 [2, TC].
            # Everything stays in [expert, token] orientation; the partition-dim
            # reduction over the 8 experts is a ones-vector matmul.
            ones8 = sgp.tile([N_EXPERTS, 1], f32, tag="ones8", bufs=1)
            nc.vector.memset(ones8, 1.0)
            plT = ps.tile([N_EXPERTS, TC], f32, tag="pg", bufs=2)
            for dt in range(DT):
                nc.tensor.matmul(
                    plT, lhsT=wr_sb[:, dt, :], rhs=xf_sb[:, dt, :],
                    start=(dt == 0), stop=(dt == DT - 1),
                )
            prod1 = sgp.tile([N_EXPERTS, TC], f32, tag="prod1", bufs=1)
            nc.vector.tensor_mul(prod1, plT, mm_sb[:, 0:TC])
            prod2 = sgp.tile([N_EXPERTS, TC], f32, tag="prod2", bufs=1)
            nc.vector.tensor_mul(prod2, plT, mm_sb[:, TC : 2 * TC])
            plv1 = ps.tile([1, TC], f32, tag="pg", bufs=2)
            nc.tensor.matmul(plv1, lhsT=ones8, rhs=prod1, start=True, stop=True)
            plv2 = ps.tile([1, TC], f32, tag="pu", bufs=2)
            nc.tensor.matmul(plv2, lhsT=ones8, rhs=prod2, start=True, stop=True)
            lv1 = sgp.tile([1, TC], f32, tag="lv1", bufs=1)
            nc.vector.tensor_copy(lv1, plv1)
            d21 = sgp.tile([1, TC], f32, tag="d21", bufs=1)
            nc.vector.tensor_sub(d21, plv2, lv1)
            ex = sgp.tile([1, TC], f32, tag="ex", bufs=1)
            nc.scalar.activation(ex, d21, AF.Exp)
            den = sgp.tile([1, TC], f32, tag="den", bufs=1)
            nc.vector.tensor_scalar(den, ex, scalar1=1.0, scalar2=None, op0=OP.add)
            w12T = sgp.tile([1, 2 * TC], f32, tag="w12T", bufs=1)
            nc.vector.reciprocal(w12T[:, 0:TC], den)
            nc.vector.tensor_mul(w12T[:, TC : 2 * TC], ex, w12T[:, 0:TC])
            nc.sync.dma_start(out=w12.ap(), in_=w12T)

            # ---- mm2 (transposed): yT[d, slot] = sum_f wd[f, d] * h[f, slot]
            yT_sb = yp.tile([P, DT, cap], bf16, tag="yT", bufs=1)
            for dt in range(DT):
                for ci, (c0, cw) in enumerate(ck1):
                    pyt = ps.tile([P, 512], f32, tag="py", bufs=2)
                    for ft in range(FT):
                        nc.tensor.matmul(
                            pyt[:, :cw],
                            lhsT=wd_sb[:, ft, dt * P : (dt + 1) * P],
                            rhs=h_sb[:, ft, c0 : c0 + cw],
                            start=(ft == 0), stop=(ft == FT - 1),
                        )
                    nc.vector.tensor_copy(yT_sb[:, dt, c0 : c0 + cw], pyt[:, :cw])
                nc.sync.dma_start(out=y_3[:, dt, :], in_=yT_sb[:, dt, :])

    nc.finalize()
    return nc


# --------------------------------------------------------------------------
# Launch 2: elementwise combine + LayerNorm. The host orders the y rows as
# two token-ordered blocks (top-1 rows, top-2 rows) so the combine is
# out[t] = w1[t]*y1[t] + w2[t]*y2[t] -- no scatter matmuls needed. All values
# (y, w1, w2) are device-computed in L1; the host only permutes them.
# --------------------------------------------------------------------------
def _build_l2(affine):
    import concourse.bacc as bacc
    import concourse.mybir as mybir
    import concourse.tile as tile

    f32 = mybir.dt.float32
    bf16 = mybir.dt.bfloat16
    AF = mybir.ActivationFunctionType
    OP = mybir.AluOpType

    TT = TC // P  # 4 token tiles

    nc = bacc.Bacc()
    yct = nc.dram_tensor("yct", [P, 2 * TT * D_MODEL], bf16, kind="ExternalInput")
    meta = nc.dram_tensor("meta", [P, 2 * TT], f32, kind="ExternalInput")
    if affine:
        gam = nc.dram_tensor("gam", [D_MODEL], bf16, kind="ExternalInput")
        bet = nc.dram_tensor("bet", [D_MODEL], bf16, kind="ExternalInput")
    out = nc.dram_tensor("out", [P, TT * D_MODEL], f32, kind="ExternalOutput")

    yct_3 = yct.rearrange("p (rc d) -> p rc d", rc=2 * TT)
    out_3 = out.rearrange("p (tt d) -> p tt d", tt=TT)

    with tile.TileContext(nc) as tc:
        with (
            tc.tile_pool(name="consts", bufs=1) as consts,
            tc.tile_pool(name="rtr", bufs=2) as rtr,
            tc.tile_pool(name="ycp", bufs=1) as ycp,
            tc.tile_pool(name="outp", bufs=1) as outp,
        ):
            meta_sb = consts.tile([P, 2 * TT], f32)
            nc.sync.dma_start(out=meta_sb, in_=meta.ap())
            yc_sb = ycp.tile([P, 2 * TT, D_MODEL], bf16)
            for tt in range(TT):  # tt-block pairs first so tt=0 starts earliest
                nc.sync.dma_start(out=yc_sb[:, tt, :], in_=yct_3[:, tt, :])
                nc.sync.dma_start(out=yc_sb[:, TT + tt, :], in_=yct_3[:, TT + tt, :])
            if affine:
                import concourse.bass as bass
                gam_sb = consts.tile([P, D_MODEL], bf16)
                bet_sb = consts.tile([P, D_MODEL], bf16)
                nc.sync.dma_start(
                    out=gam_sb,
                    in_=bass.AP(tensor=gam.ap().tensor, offset=0, ap=[[0, P], [1, D_MODEL]]),
                )
                nc.sync.dma_start(
                    out=bet_sb,
                    in_=bass.AP(tensor=bet.ap().tensor, offset=0, ap=[[0, P], [1, D_MODEL]]),
                )
            eps_sb = consts.tile([P, 1], f32)
            nc.vector.memset(eps_sb, LN_EPS)

            # per-tt chain: ACT does scale-mul + sqrt, DVE does fused
            # mul-add, bn_stats and normalize
            o_sb = outp.tile([P, TT, D_MODEL], f32, tag="acc", bufs=1)
            for tt in range(TT):
                t1 = rtr.tile([P, D_MODEL], f32, tag="t1")
                nc.scalar.activation(
                    t1, yc_sb[:, tt, :], AF.Copy, bias=0.0,
                    scale=meta_sb[:, tt : tt + 1],
                )
                a = o_sb[:, tt, :]
                nc.vector.scalar_tensor_tensor(
                    a, yc_sb[:, TT + tt, :], meta_sb[:, TT + tt : TT + tt + 1], t1,
                    op0=OP.mult, op1=OP.add,
                )
                a2 = a.rearrange("p (s f) -> p s f", s=2)
                stats = rtr.tile([P, 2, 6], f32, tag="stats")
                for s_ in range(2):
                    nc.vector.bn_stats(out=stats[:, s_, :], in_=a2[:, s_, :])
                mv = rtr.tile([P, 2], f32, tag="mv")
                nc.vector.bn_aggr(out=mv, in_=stats)
                mean = mv[:, 0:1]
                rstd = rtr.tile([P, 1], f32, tag="rstd")
                nc.scalar.activation(
                    rstd, mv[:, 1:2], AF.Sqrt, bias=eps_sb, scale=1.0, alpha=0.0
                )
                nc.vector.reciprocal(rstd, rstd)
                of = rtr.tile([P, D_MODEL], f32, tag="of")
                nc.vector.tensor_scalar(
                    of, a, scalar1=mean, scalar2=rstd,
                    op0=OP.subtract, op1=OP.mult,
                )
                if affine:
                    nc.vector.tensor_mul(of, of, gam_sb)
                    nc.vector.tensor_add(of, of, bet_sb)
                nc.sync.dma_start(out=out_3[:, tt, :], in_=of)

    nc.finalize()
    return nc


# --------------------------------------------------------------------------
# Host orchestration
# --------------------------------------------------------------------------
def _route(x2, w_router):
    logits = x2 @ w_router.T
    order = np.argsort(-logits, axis=1)
    top1 = order[:, 0].astype(np.int64)
    top2 = order[:, 1].astype(np.int64)
    return top1, top2


def _prepare(inputs):
    bf = ml_dtypes.bfloat16
    x2 = np.ascontiguousarray(
        np.asarray(inputs["x"], dtype=np.float32).reshape(T_FULL, D_MODEL)
    )
    w_router = np.asarray(inputs["w_router"], dtype=np.float32)
    top1, top2 = _route(x2, w_router)

    # per-expert token lists (ascending)
    tok = [np.where((top1 == e) | (top2 == e))[0] for e in range(N_EXPERTS)]
    caps = [len(t) for t in tok]
    cap_needed = max(caps)
    return x2, w_router, top1, top2, tok, caps, cap_needed


def _pm(a, inner, width):
    """[ (g p), w ] row-major -> partition-major [P, g*w] contiguous rows."""
    g = a.shape[0] // P
    return np.ascontiguousarray(
        a.reshape(g, P, inner, width).transpose(1, 0, 2, 3).reshape(P, -1)
        if inner > 1 else
        a.reshape(g, P, width).transpose(1, 0, 2).reshape(P, -1)
    )


def _l1_in_maps(inputs, x2, w_router, top1, top2, tok, cap):
    bf = ml_dtypes.bfloat16
    nck1, ck1 = _mm1_chunks(cap)
    w_gate = np.asarray(inputs["w_gate"], dtype=np.float32)
    w_up = np.asarray(inputs["w_up"], dtype=np.float32)
    w_down = np.asarray(inputs["w_down"], dtype=np.float32)
    # wrt: [d, e] -> [P, dt*e] partition-major
    wrt = np.ascontiguousarray(
        w_router.T.reshape(DT, P, N_EXPERTS).transpose(1, 0, 2).reshape(P, -1)
    ).astype(bf)
    m1_full = np.zeros((N_EXPERTS, T_FULL), np.float32)
    m1_full[top1, np.arange(T_FULL)] = 1.0
    m2_full = np.zeros((N_EXPERTS, T_FULL), np.float32)
    m2_full[top2, np.arange(T_FULL)] = 1.0
    in_maps = []
    for e in range(N_EXPERTS):
        # xg: [P, nck1, DT, 512] partition-major, chunk blocks padded to 512
        xgT = np.zeros((D_MODEL, cap), np.float32)
        xgT[:, : len(tok[e])] = x2[tok[e]].T
        xgT_r = xgT.reshape(DT, P, cap)
        xg4 = np.empty((P, DT * cap), np.float32)
        for ci, (c0, cw) in enumerate(ck1):
            xg4[:, DT * c0 : DT * (c0 + cw)] = (
                xgT_r[:, :, c0 : c0 + cw].transpose(1, 0, 2).reshape(P, -1)
            )
        # wg/wu: [(dt p), f] -> [P, fs, dt, 512] -> rows
        wgT = w_gate[e].T.reshape(DT, P, 8, 256)
        wuT = w_up[e].T.reshape(DT, P, 8, 256)
        wg4 = wgT.transpose(1, 2, 0, 3).reshape(P, -1)
        wu4 = wuT.transpose(1, 2, 0, 3).reshape(P, -1)
        # wd: [(ft p), d] -> [P, ft, d] -> rows
        wd3 = w_down[e].T.reshape(FT, P, D_MODEL).transpose(1, 0, 2).reshape(P, -1)
        lo, hi = e * TC, (e + 1) * TC  # this core also routes token block e
        xf3 = x2[lo:hi].T.reshape(DT, P, TC).transpose(1, 0, 2).reshape(P, -1)
        in_maps.append({
            "xgt": np.ascontiguousarray(xg4).astype(bf),
            "wgt": np.ascontiguousarray(wg4).astype(bf),
            "wut": np.ascontiguousarray(wu4).astype(bf),
            "wdt": np.ascontiguousarray(wd3).astype(bf),
            "xtf": np.ascontiguousarray(xf3).astype(bf),
            "wrt": wrt,
            "mmh": np.ascontiguousarray(
                np.concatenate([m1_full[:, lo:hi], m2_full[:, lo:hi]], axis=1)
            ),
        })
    return in_maps


def _l2_in_maps(inputs, top1, top2, tok, y_parts, w12_parts, affine):
    bf = ml_dtypes.bfloat16
    TT = TC // P
    in_maps = []
    for c in range(N_CORES):
        lo, hi = c * TC, (c + 1) * TC
        y1 = np.empty((TC, D_MODEL), y_parts[0].dtype)
        y2 = np.empty((TC, D_MODEL), y_parts[0].dtype)
        t1c = top1[lo:hi]
        t2c = top2[lo:hi]
        for e in range(N_EXPERTS):
            m = t1c == e
            if m.any():
                y1[m] = y_parts[e][np.searchsorted(tok[e], np.nonzero(m)[0] + lo)]
            m = t2c == e
            if m.any():
                y2[m] = y_parts[e][np.searchsorted(tok[e], np.nonzero(m)[0] + lo)]
        yct = np.concatenate([y1, y2], axis=0)
        meta = np.empty((P, 2 * TT), np.float32)
        meta[:, :TT] = w12_parts[c][0].reshape(TT, P).T
        meta[:, TT:] = w12_parts[c][1].reshape(TT, P).T
        in_map = {
            "yct": np.ascontiguousarray(
                yct.reshape(2 * TT, P, D_MODEL).transpose(1, 0, 2).reshape(P, -1)
            ),
            "meta": meta,
        }
        if affine:
            in_map["gam"] = np.asarray(inputs["ln_gamma"], np.float32).astype(bf)
            in_map["bet"] = np.asarray(inputs["ln_beta"], np.float32).astype(bf)
        in_maps.append(in_map)
    return in_maps


def run_launches(inputs, trace=False):
    from concourse.bass_utils import run_bass_kernel_spmd

    x2, w_router, top1, top2, tok, caps, cap_needed = _prepare(inputs)
    cap = _CACHED.get("cap", CAP_DEFAULT)
    if cap_needed > cap:
        cap = int(-(-cap_needed // 8) * 8)
        _CACHED.pop("l1", None)
    affine = not (
        np.all(np.asarray(inputs["ln_gamma"]) == 1.0)
        and np.all(np.asarray(inputs["ln_beta"]) == 0.0)
    )
    if "l1" not in _CACHED or _CACHED.get("cap") != cap:
        _CACHED["cap"] = cap
        _CACHED["l1"] = _build_l1(cap)
    if "l2" not in _CACHED or _CACHED.get("affine") != affine:
        _CACHED["affine"] = affine
        _CACHED["l2"] = _build_l2(affine)

    l1_maps = _l1_in_maps(inputs, x2, w_router, top1, top2, tok, cap)
    res1 = run_bass_kernel_spmd(
        _CACHED["l1"], l1_maps, core_ids=list(range(N_CORES)), trace=trace
    )
    cap = _CACHED["cap"]
    y_parts = [
        np.asarray(res1.results[e]["y"]).reshape(P, DT, cap).transpose(2, 1, 0).reshape(cap, D_MODEL)
        for e in range(N_EXPERTS)
    ]
    w12_parts = [
        np.asarray(res1.results[c]["w12"]).reshape(2, TC) for c in range(N_CORES)
    ]

    l2_maps = _l2_in_maps(inputs, top1, top2, tok, y_parts, w12_parts, affine)
    res2 = run_bass_kernel_spmd(
        _CACHED["l2"], l2_maps, core_ids=list(range(N_CORES)), trace=trace
    )
    outs = []
    for c in range(N_CORES):
        o = np.asarray(res2.results[c]["out"]).reshape(P, TT_L2, D_MODEL)
        outs.append(o.transpose(1, 0, 2).reshape(TC, D_MODEL))
    out = np.concatenate(outs, axis=0)
    return out.reshape(B, S, D_MODEL), res1, res2


def kernel(**inputs) -> np.ndarray:
    out, _, _ = run_launches(inputs, trace=False)
    return out
